# revision 19
# baseline (speedup 1.0000x reference)
"""IntLUTConv (1x1 conv as per-pixel GEMM) on 8 TRN2 NeuronCores.

Sharding: data-parallel over batch (B=8 -> one batch item per core), no
collectives. Per core: 16.8 MB fp32 in + 4.2 MB int8 out, streamed at
the per-core DMA ceiling, plus a fixed ~9 us end-of-NEFF tax (NRT
injects a serial reset of the full 256-entry semaphore file into every
NEFF; measured identical for a trivial 2-DMA kernel) that bounds any
kernel on this stack. Design, all choices A/B-measured on HW:

  host packs x chunk-major: xh[p, 2*off + ct*csz + px], so each
  (chunk, partition) is ONE 8 KB contiguous DRAM run
  -> input chunks ride a SINGLE HWDGE ring (sync) as one 128-descriptor
     DMA each. Single-ring beat dual-ring by ~4 us: two rings split
     engine bandwidth so the first chunk completes ~2x later and that
     pipeline-fill lag is repaid in the drain; fewer+bigger descriptors
     also sustain ~415 GB/s/core vs ~360 with 256x4KB
  -> fused clip+trunc+quantize in ONE 7-op custom DVE pass
     (sign-aware pre-shift + magic-constant RNE; exact except for 31
      fp32 bit patterns out of 2^30, each off by one quant step)
  -> fp8e4 flat tile, viewed [128, 2, csz] -> DoubleRow matmuls
     (K=256, N=512 = one PSUM bank per matmul, fp32 PSUM; exact
     integer math). N=512 halves the Ldweights count vs N=256 (the
     walrus ldw-dedup pass is disabled, every matmul reloads weights)
  -> ACT Copy(scale=scale/64, bias=offset) PSUM->SBUF int8 (hardware
     cast is round-half-even + saturate, exactly matching
     clip(round(y*scale/64 + offset), -128, 127))
  -> int8 stores, chunk-major (un-permuted on host), ONE fused
     [128, 2, csz] DMA per chunk on the act ring; the tapered tail
     chunks' stores drain on the sync ring, idle once input ends.

Measured (NTFF, min of 5): 65.4 us single-core, ~65-67 us on 8-core
SPMD, vs 75.9 us for the previous baseline. Remaining time: ~51 us
stream floor + ~2.5 us start ramp + ~3 us drain + ~9 us fixed NRT
semaphore sweep + barrier.
"""
import re
import numpy as np

import concourse.bacc as bacc
import concourse.tile as tile
import concourse.mybir as mybir
from concourse.bass_utils import run_bass_kernel_spmd
from concourse.dve_spec import (
    Spec, Src0, Zero, C0, C1, C2, C3, maxx, minn, select,
    _spill_c3_to_src1,
)
from concourse.dve_ops import OPS, DveOp

B, CIN, COUT, H, W = 8, 256, 256, 128, 128
NPX = H * W            # 16384 pixels per batch item
F = 1024               # max pixel piece per quantize/matmul stage
MAGIC = 12582912.0     # 1.5 * 2**23: float add forces RNE to integer grid
CHALF = 0.4999995231628418   # 0.5 - 2**-21: pre-shift for trunc-via-RNE
UBND = 6.500000476837158     # 7 - CHALF: upper clip bound post-shift
LBND = -7.500000476837158    # -8 + CHALF: lower clip bound post-shift

# chunk schedule: steady 1024s with a 512 taper at the end — the post-
# last-input drain is a serial quantize->matmul->copy->store chain whose
# length scales with the final chunks' size. (A front ramp and deeper
# tapers measured slower: per-chunk fixed costs dominate below 512.)
CSIZES = [1024] * 14 + [512] * 4
assert sum(CSIZES) == NPX

TRACE = False          # test.py sets True to collect NTFF exec time
_LAST_RESULTS = [None]


def _truncq_ref(in0, in1, s0, s1, imm2):
    return np.trunc(np.clip(in0, -8.0, 7.0))


def _register_truncq():
    """clip(-8,7) + trunc-toward-zero fused in one 7-op DVE pass.

    h   = copysign(0.5 - 2^-21, x)         (sign-aware pre-shift)
    y   = clamp(x - h, -7.5000005, 6.5000005)
    out = (y + MAGIC) - MAGIC              (RNE to integer grid == trunc)

    Shift-then-clip: any x >= 7 lands on the upper bound (rounds to 7),
    any x <= -8 on the lower (rounds to -8). Exact vs trunc(clip(x)) for
    all fp32 except 31 bit patterns of the form (integer - tiny), each
    off by one quantization step (brute-force verified over all fp32 in
    [-16,16]; ~1e-7 incidence for this input distribution).
    """
    for existing in OPS:
        if existing.name == "TRUNCQC_ANT":
            return existing
    c = Src0 < Zero
    h = select(c, Zero - C1, C1)   # +/-CHALF; Zero-C1 hoists to a Latch
    y = Src0 - h
    y1 = minn(y, C2)               # C2 = UBND (imm2)
    y2 = maxx(y1, C3)              # C3 = LBND (spilled to in1)
    body = (y2 + C0) - C0          # C0 = MAGIC (s0)
    body = _spill_c3_to_src1(body)
    op = DveOp("TRUNCQC_ANT", Spec(body=body, reference=_truncq_ref),
               subdim=False, uops_sha={})
    OPS.append(op)
    import concourse.dve_ops as dve_ops_mod
    dve_ops_mod.CUSTOM_DVE_SPECS[op.name] = op.spec
    dve_ops_mod._SUB_OPCODE_FOR_NAME[op.name] = (
        dve_ops_mod._CUSTOM_DVE_ROW_BASE + len(OPS) - 1)
    assert dve_ops_mod._SUB_OPCODE_FOR_NAME[op.name] < 0x20
    try:
        op.compile("v3")
    except ValueError as e:
        m = re.search(r'uops_sha\["v3"\]="([0-9a-f]+)"', str(e))
        if not m:
            raise
        op.uops_sha["v3"] = m.group(1)
        op.compile("v3")
    return op


def _ranges(sizes):
    out, off = [], 0
    for sz in sizes:
        out.append((off, sz))
        off += sz
    return out


def _build(scale_val: float, offset_val: float,
           csizes=CSIZES, F: int = F,
           work_bufs: int = 4, xq_bufs: int = 6, out_bufs: int = 6,
           ps_bufs: int = 4,
           in_rings=("sync", "scalar"), in_mode: str = "paired",
           out_ring: str = "gpsimd",
           tail_on_sync: int = 3, mm_n: int = 256,
           merged_copy: bool = False, fuse_out: bool = False,
           flat_in: bool = False, prime: bool = False):
    op = _register_truncq()
    nc = bacc.Bacc("TRN2", target_bir_lowering=False)
    # chunk-major host layouts: column base of chunk c is 2*off; within a
    # chunk, ct (input) / o (output) halves are adjacent per partition
    x = nc.dram_tensor("x", [128, 2 * NPX], mybir.dt.float32,
                       kind="ExternalInput")
    # wt3[o][p][ki][m] = W[o*128+m, ki*128+p]
    wt3 = nc.dram_tensor("wt3", [2, 128, 2, 128], mybir.dt.float8e4,
                         kind="ExternalInput")
    out = nc.dram_tensor("out", [128, 2 * NPX], mybir.dt.int8,
                         kind="ExternalOutput")

    max_chunk = max(csizes)
    n_chunks = len(csizes)

    with tile.TileContext(nc) as tc, \
         tc.tile_pool(name="singles", bufs=1) as singles, \
         tc.tile_pool(name="work", bufs=work_bufs) as work, \
         tc.tile_pool(name="xqp", bufs=xq_bufs) as xqp, \
         tc.tile_pool(name="outs", bufs=out_bufs) as outs, \
         tc.tile_pool(name="psum", bufs=ps_bufs, space="PSUM") as pspool:
        if prime:
            # 1-descriptor dummy load: arms the input ring so its arm
            # latency overlaps chunk 0's descriptor writing
            pr = singles.tile([1, 1], mybir.dt.float32, tag="prime")
            getattr(nc, in_rings[0]).dma_start(out=pr[:, :], in_=x[:1, :1])
        wt_sb = []
        for o in range(2):
            w_t = singles.tile([128, 2, 128], mybir.dt.float8e4, tag=f"wt{o}")
            nc.scalar.dma_start(out=w_t[:, :, :], in_=wt3[o, :, :, :])
            wt_sb.append(w_t)
        lb = singles.tile([128, 1], mybir.dt.float32, tag="lb")
        nc.vector.memset(lb[:, :], LBND)

        for idx, (coff, csz) in enumerate(_ranges(csizes)):
            base = 2 * coff
            tail = idx >= n_chunks - tail_on_sync
            # fixed tile shapes regardless of chunk size: uniform tags keep
            # the Tile semaphore set (and its serial end-of-kernel reset
            # sweep) small
            if in_mode == "alt":
                eng = getattr(nc, in_rings[idx % len(in_rings)])
                pair = (eng, eng)
            elif in_mode == "paired_after2" and idx < 2:
                eng = getattr(nc, in_rings[0])
                pair = (eng, eng)
            else:  # "paired": two rings write descriptors in parallel
                pair = (getattr(nc, in_rings[0]), getattr(nc, in_rings[1]))
            if flat_in:
                # one DMA per chunk: 128 descriptors of 2*csz*4-byte runs
                assert csz <= F
                xrf = work.tile([128, 2 * max_chunk], mybir.dt.float32,
                                tag="xr")
                pair[0].dma_start(out=xrf[:, :2 * csz],
                                  in_=x[:, base:base + 2 * csz])
            else:
                xr = work.tile([128, 2, max_chunk], mybir.dt.float32,
                               tag="xr")
                # each ct half: one 128-descriptor DMA of csz*4-byte runs
                pair[0].dma_start(out=xr[:, 0, :csz],
                                  in_=x[:, base:base + csz])
                pair[1].dma_start(out=xr[:, 1, :csz],
                                  in_=x[:, base + csz:base + 2 * csz])
            for boff, bsz in _ranges([F] * (csz // F) if csz >= F else [csz]):
                if flat_in:
                    xqf = xqp.tile([128, 2 * F], mybir.dt.float8e4, tag="xq")
                    nc.vector._custom_dve(op, out=xqf[:, :2 * bsz],
                                          in0=xrf[:, :2 * bsz],
                                          in1=lb[:, :], s0=MAGIC, s1=CHALF,
                                          imm2=UBND)
                    xq = xqf[:, :2 * bsz].rearrange("p (c n) -> p c n", c=2)
                else:
                    xqt = xqp.tile([128, 2, F], mybir.dt.float8e4, tag="xq")
                    nc.vector._custom_dve(op, out=xqt[:, :, :bsz],
                                          in0=xr[:, :, boff:boff + bsz],
                                          in1=lb[:, :], s0=MAGIC, s1=CHALF,
                                          imm2=UBND)
                    xq = xqt
                oc = outs.tile([128, 2, F], mybir.dt.int8, tag="oc")
                n = min(mm_n, bsz)
                subs = [n] * (bsz // n) + ([bsz % n] if bsz % n else [])
                if merged_copy:
                    ps2 = pspool.tile([128, 2, F], mybir.dt.float32, tag="ps")
                    for o in range(2):
                        for soff, ssz in _ranges(subs):
                            nc.tensor.matmul(
                                ps2[:, o, soff:soff + ssz],
                                wt_sb[o][:, :, :],
                                xq[:, :, soff:soff + ssz],
                                start=True, stop=True,
                                perf_mode=mybir.MatmulPerfMode.DoubleRow,
                            )
                    nc.scalar.activation(
                        out=oc[:, :, :bsz], in_=ps2[:, :, :bsz],
                        func=mybir.ActivationFunctionType.Copy,
                        scale=scale_val / 64.0, bias=offset_val,
                    )
                else:
                    for o in range(2):
                        ps = pspool.tile([128, F], mybir.dt.float32, tag="ps")
                        for soff, ssz in _ranges(subs):
                            nc.tensor.matmul(
                                ps[:, soff:soff + ssz],
                                wt_sb[o][:, :, :],
                                xq[:, :, soff:soff + ssz],
                                start=True, stop=True,
                                perf_mode=mybir.MatmulPerfMode.DoubleRow,
                            )
                        nc.scalar.activation(
                            out=oc[:, o, :bsz], in_=ps[:, :bsz],
                            func=mybir.ActivationFunctionType.Copy,
                            scale=scale_val / 64.0, bias=offset_val,
                        )
                # chunk-major output: o halves adjacent per partition
                out_eng = nc.sync if tail else getattr(nc, out_ring)
                if fuse_out and bsz == csz:
                    # one DMA per chunk: [128, 2, csz] AP, DRAM o-halves
                    # adjacent so each partition is 2*csz contiguous
                    ov_c = out[:, base:base + 2 * csz].rearrange(
                        "p (o n) -> p o n", o=2)
                    out_eng.dma_start(out=ov_c[:, :, :], in_=oc[:, :, :bsz])
                else:
                    for o in range(2):
                        out_eng.dma_start(
                            out=out[:, base + o * csz + boff:
                                    base + o * csz + boff + bsz],
                            in_=oc[:, o, :bsz])
    nc.finalize()
    return nc


_KERNEL_CACHE: dict = {}


def _weights_host(weights: np.ndarray) -> np.ndarray:
    dt_f8 = mybir.dt.np(mybir.dt.float8e4)
    w4 = weights.reshape(2, 128, 2, 128)          # [o, m, ki, p]
    wt3 = np.ascontiguousarray(w4.transpose(0, 3, 2, 1))  # [o, p, ki, m]
    return wt3.astype(np.float32).astype(dt_f8)


def _pack_x(xb: np.ndarray) -> np.ndarray:
    """[256, NPX] fp32 -> chunk-major [128, 2*NPX]:
    xh[p, 2*off + ct*csz + px] = xb[ct*128 + p, off + px]"""
    x2 = xb.reshape(2, 128, NPX)
    parts = [x2[:, :, off:off + csz].transpose(1, 0, 2).reshape(128, 2 * csz)
             for off, csz in _ranges(CSIZES)]
    return np.ascontiguousarray(np.concatenate(parts, axis=1))


def _unpack_out(oh: np.ndarray) -> np.ndarray:
    """chunk-major [128, 2*NPX] int8 -> [COUT, NPX]:
    out[o*128 + p, off + px] = oh[p, 2*off + o*csz + px]"""
    o2 = np.empty((2, 128, NPX), dtype=np.int8)
    for off, csz in _ranges(CSIZES):
        blk = oh[:, 2 * off:2 * off + 2 * csz].reshape(128, 2, csz)
        o2[:, :, off:off + csz] = blk.transpose(1, 0, 2)
    return o2.reshape(COUT, NPX)


def kernel(x, weights, scale, offset):
    x = np.asarray(x)
    weights = np.asarray(weights)
    sv = float(np.asarray(scale))
    ov = float(np.asarray(offset))

    key = (sv, ov)
    if key not in _KERNEL_CACHE:
        _KERNEL_CACHE[key] = _build(
            sv, ov, in_rings=("sync",), in_mode="alt", out_ring="scalar",
            csizes=CSIZES, mm_n=512, work_bufs=5, xq_bufs=4, out_bufs=4,
            ps_bufs=4, fuse_out=True, flat_in=True, tail_on_sync=5)
    nc = _KERNEL_CACHE[key]

    wt_host = _weights_host(weights)
    in_maps = [
        {"x": _pack_x(x[b].reshape(CIN, NPX)), "wt3": wt_host}
        for b in range(B)
    ]
    res = run_bass_kernel_spmd(nc, in_maps, core_ids=list(range(B)),
                               trace=TRACE)
    _LAST_RESULTS[0] = res
    return np.stack([_unpack_out(r["out"]).reshape(COUT, H, W)
                     for r in res.results])
